# revision 26
# baseline (speedup 1.0000x reference)
"""Trainium2 Bass kernel for nn_NeuralDecisionTree.

Strategy (data-parallel over batch, 8 cores):
  reference:  x = features @ mask.T            [B, 1024]   (one-hot row select)
              d = sigmoid(x @ W + b)           [B, 1024]
              mu = tree-routing products       [B, 1024]
              out = mu @ softmax(pi)           [B, 100]

  The mask matmul is an exact column-selection, folded into W on the host.
  The host also pre-transposes/gathers features into [feature, batch] chunk
  layout and quantizes both matmul operands to fp8 e4m3 (W scaled by 16,
  descaled inside the fused sigmoid), so the device main matmul runs in
  DoubleRow fp8 mode: each MM contracts 256 features (two 128-row chunks
  packed per PE cell) at ~2x bf16 throughput.

    zT[s, b]  = sum_f W2p[f, s] * featT[f, b]             (PE, fp8 DoubleRow)
    d         = sigmoid(zT/16 + b)                        (ACT, bf16 out)
    mu        = 10 levels of routing products             (DVE, bf16;
                right child = mu - mu*d, so no second sigmoid is needed)
    yT[c, b]  = sum_s probsP[s, c] * mu10[s, b]           (PE, bf16)

  Tree levels 0-6 (the "phase A" that previously needed two DMA-xbar
  transposes and a [batch, path] pass) are instead computed DIRECTLY in
  [path, batch] layout: with the slot permutation below, the level-l
  ancestor of path p lives at slot 2^l + (p mod 2^l), so one matmul with a
  constant +-1 selection matrix per level expands d-tile0 into
  E_l[p, b] = d(anc) or 1-d(anc)   (slot 0 of d0 is pinned to the
  constant 1.0 via a large bias on a zeroed weight column, providing the
  "1" in 1-d).  mu7 = prod_l E_l is a running DVE product.  Levels 7-9
  use contiguous d slices in [slot, b] layout with the mul/sub trick, and
  the leaf matmul consumes mu10 directly.  Every block is self-contained
  (no cross-block transpose/phase chains, no DMA_TRANSPOSE drains), so
  the PE stream never waits at block boundaries and the tail after the
  last main MM is just sig+mul+mm+sub+mm+copy+store.
"""

import ml_dtypes
import numpy as np

import concourse.bass as bass  # noqa: F401
import concourse.mybir as mybir
import concourse.tile as tile
from concourse import bacc
from concourse.bass_utils import run_bass_kernel_spmd

F32 = mybir.dt.float32
BF16 = mybir.dt.bfloat16
FP8 = mybir.dt.float8e4

B = 16384
NCORES = 8
BC = B // NCORES      # 2048 batch rows per core
SG = 512              # batch rows processed end-to-end per block
NSG = BC // SG        # 4
NF = 1024             # used features (host gathers mask-selected columns)
NL = 1024             # tree nodes / leaves / dense units
NCLS = 100            # classes
KCH = NF // 128       # 8 contraction chunks of 128
NDR = KCH // 2        # 4 double-row chunks of 256
NT = NL // 128        # 8 slot tiles
WSCALE = 16.0         # host premultiplies W2 by this; sigmoid descales
NWARM = 44            # PE warm-up matmuls covering the head DMA wait

# test.py can override (e.g. {"trace": True}) and read LAST_RESULT
RUN_KWARGS: dict = {}
LAST_RESULT = None


def _bitrev(q: int, bits: int) -> int:
    r = 0
    for m in range(bits):
        if (q >> m) & 1:
            r |= 1 << (bits - 1 - m)
    return r


def _node_of_slot() -> np.ndarray:
    """slot -> original node id. Slots are laid out so the level-l ancestor
    of path p sits at slot 2^l + (p mod 2^l) (tile 0) and levels 7-9 each
    occupy contiguous aligned tiles."""
    node = np.zeros(NL, dtype=np.int64)
    node[0] = 0  # pinned to constant 1.0 on device (bias trick)
    for l in range(7):
        for q in range(1 << l):
            node[(1 << l) + q] = (1 << l) + _bitrev(q, l)
    for q7 in range(128):
        node[128 + q7] = 128 + _bitrev(q7, 7)
    for j1 in range(2):
        for q7 in range(128):
            node[256 + j1 * 128 + q7] = 256 + 2 * _bitrev(q7, 7) + j1
    for j2 in range(4):
        c7, c8 = j2 & 1, j2 >> 1
        for q7 in range(128):
            node[512 + j2 * 128 + q7] = 512 + 4 * _bitrev(q7, 7) + 2 * c7 + c8
    return node


def _leaf_of_row() -> np.ndarray:
    """probsP row r = j3*128 + q7 -> original leaf index."""
    L = np.zeros(NL, dtype=np.int64)
    for j3 in range(8):
        c789 = [j3 & 1, (j3 >> 1) & 1, (j3 >> 2) & 1]
        for q7 in range(128):
            c = [(q7 >> m) & 1 for m in range(7)] + c789
            L[j3 * 128 + q7] = sum(c[m] << (9 - m) for m in range(10))
    return L


def _expansion_mats() -> np.ndarray:
    """S[l][slot, p] so that (S[l].T @ d0)[p] = d(anc_l(p)) if bit l of p
    is 0 else 1 - d(anc_l(p)), using d0[slot 0] == 1."""
    S = np.zeros((7, 128, 128), dtype=np.float32)
    for l in range(7):
        for p in range(128):
            c_l = (p >> l) & 1
            slot = (1 << l) + (p & ((1 << l) - 1))
            S[l][slot, p] = -1.0 if c_l else 1.0
            if c_l:
                S[l][0, p] += 1.0
    return S


def _build_program():
    nc = bacc.Bacc("TRN2", target_bir_lowering=False)
    feat = nc.dram_tensor("feat", [128, NSG * KCH * SG], FP8, kind="ExternalInput")
    w2p = nc.dram_tensor("w2p", [128, NT * NF], FP8, kind="ExternalInput")
    biases = nc.dram_tensor("biases", [128, 2 * NT], F32, kind="ExternalInput")
    pip = nc.dram_tensor("pip", [128, NT * NCLS], BF16, kind="ExternalInput")
    sxp = nc.dram_tensor("sxp", [128, 7 * 128], BF16, kind="ExternalInput")
    yT = nc.dram_tensor("yT", [NCLS, BC], F32, kind="ExternalOutput")

    SIG = mybir.ActivationFunctionType.Sigmoid
    DR = mybir.MatmulPerfMode.DoubleRow
    SGB = KCH * SG  # fp8 bytes per sg slice of feat, per partition
    QB = 2 * SG     # fp8 bytes per DR-chunk quarter, per partition

    with tile.TileContext(nc) as tc:
        with (
            tc.tile_pool(name="const", bufs=1) as cpool,
            tc.tile_pool(name="featT", bufs=4) as ftpool,
            tc.tile_pool(name="dsig", bufs=2) as dpool,
            tc.tile_pool(name="mu", bufs=2) as mupool,
            tc.tile_pool(name="outst", bufs=2) as opool,
            tc.tile_pool(name="pz", bufs=3, space="PSUM") as pz,
            tc.tile_pool(name="pe", bufs=4, space="PSUM") as pe,
            tc.tile_pool(name="py", bufs=1, space="PSUM") as py,
        ):
            def load_ft(sg):
                """One dma_start for the whole sg slice (one SP issue slot);
                quarters are views into the one tile."""
                big = ftpool.tile([128, SGB], FP8, tag="ftbig")
                nc.sync.dma_start(big, feat[:, sg * SGB:(sg + 1) * SGB])
                return [big[:, c * QB:(c + 1) * QB] for c in range(NDR)]

            # ---- DMA priority order: block-0 critical loads on the SP
            # queue; w2 tiles 4-7 in parallel on the GpSimd SWDGE queue;
            # ft1/pip HELD behind a dummy read of ft0 (the 16 DMA rings
            # round-robin every in-flight transfer, so unheld they would
            # delay ft0 until the whole input set lands). ----
            w2 = cpool.tile([128, NT * NF], FP8)
            nc.sync.dma_start(w2[:, 0:NF], w2p[:, 0:NF])
            ft_bufs = {0: load_ft(0)}
            bia = cpool.tile([128, 2 * NT], F32)
            nc.sync.dma_start(bia, biases[:, :])
            sexp = cpool.tile([128, 7 * 128], BF16)
            nc.sync.dma_start(sexp, sxp[:, :])

            # everything else is HELD behind dummy GpSimd reads of the ft0
            # tile (WAR on the head of each destination), so the critical
            # w2t0+ft0 transfers get the full DMA bandwidth
            wt = cpool.tile([128, 128], BF16)
            nc.gpsimd.memset(wt, 0.0)
            ft0q = ft_bufs[0][0]
            big1 = ftpool.tile([128, SGB], FP8, tag="ftbig")
            pp = cpool.tile([128, NT * NCLS], BF16)
            nc.gpsimd.tensor_copy(w2[:, NF:NF + 4], w2[:, 0:4])
            nc.gpsimd.tensor_copy(w2[:, 4 * NF:4 * NF + 4], ft0q[:, 0:4])
            nc.gpsimd.tensor_copy(big1[:, 0:4], ft0q[:, 0:4])
            nc.gpsimd.tensor_copy(pp[:, 0:4], ft0q[:, 0:4])
            nc.sync.dma_start(w2[:, NF:4 * NF], w2p[:, NF:4 * NF])
            nc.gpsimd.dma_start(w2[:, 4 * NF:NT * NF], w2p[:, 4 * NF:NT * NF])
            nc.gpsimd.dma_start(big1, feat[:, SGB:2 * SGB])
            ft_bufs[1] = [big1[:, c * QB:(c + 1) * QB] for c in range(NDR)]
            nc.gpsimd.dma_start(pp, pip[:, :])

            # warm-up burst: keep the PE busy during the head DMA wait so
            # the HAM clock gate is at 8/8 when the first real matmuls
            # issue; the dummy sigmoid starts the ACT_TABLE_LOAD DMA before
            # the input loads saturate the DMA rings.
            wp = pz.tile([128, SG], F32, tag="z")
            for _ in range(NWARM):
                nc.tensor.matmul(wp[:, 0:128], wt, wt, start=True, stop=True)
            scrap = dpool.tile([128, 4], BF16, tag="scrap")
            nc.scalar.activation(scrap, wt[:, 0:4], SIG)

            def mm_group(t, ft):
                """One slot tile's 4-chunk DoubleRow accumulation -> zp."""
                zp = pz.tile([128, SG], F32, tag="z")
                for c in range(NDR):
                    wsl = w2[:, (t * KCH + 2 * c) * 128:
                             (t * KCH + 2 * c + 2) * 128]
                    nc.tensor.matmul(
                        zp,
                        wsl.rearrange("p (k s) -> p k s", k=2),
                        ft[c].rearrange("p (k b) -> p k b", k=2),
                        start=(c == 0), stop=(c == NDR - 1),
                        perf_mode=DR,
                    )
                return zp

            def t0_sig(ft):
                """Tile-0 matmuls + sigmoid -> d0 (incl. constant-1 slot 0)."""
                d0 = dpool.tile([128, SG], BF16, tag="d0")
                zp0 = mm_group(0, ft)
                nc.scalar.activation(
                    d0, zp0, SIG, bias=bia[:, 0:1], scale=1.0 / WSCALE
                )
                return d0

            def expand(l, d0):
                Ep = pe.tile([128, SG], F32, tag="E")
                nc.tensor.matmul(
                    Ep, sexp[:, l * 128:(l + 1) * 128], d0,
                    start=True, stop=True,
                )
                if l < 4:
                    # drain to SBUF bf16 on the ACT (it has slack); frees
                    # the E bank early and speeds the DVE product chain
                    Es = mupool.tile([128, SG], BF16, tag=f"Es{l}")
                    nc.scalar.copy(Es, Ep)
                    return Es
                return Ep

            def mu7_chain(E):
                """DVE running product over the 7 expanded levels; E[0..3]
                were drained to SBUF bf16 by the ACT, so the first muls run
                at the 334ns bf16 rate instead of the 600ns PSUM-f32 rate."""
                mu7 = mupool.tile([128, SG], BF16, tag="mu7")
                nc.vector.tensor_mul(mu7, E[0], E[1])
                for l in range(2, 7):
                    nc.vector.tensor_mul(mu7, mu7, E[l])
                return mu7

            # ---- prologue: block 0's tile0 + expansions + mu7 ----
            d0p = t0_sig(ft_bufs[0])
            Ep0 = [expand(l, d0p) for l in range(7)]
            mu7_cur = mu7_chain(Ep0)

            for sg in range(NSG):
                ft = ft_bufs.pop(sg)
                if sg + 2 < NSG:
                    ft_bufs[sg + 2] = load_ft(sg + 2)
                dsg = dpool.tile([128, 7 * SG], BF16, tag="d")

                def tile_mm(t):
                    zp = mm_group(t, ft)
                    nc.scalar.activation(
                        dsg[:, (t - 1) * SG:t * SG], zp, SIG,
                        bias=bia[:, t:t + 1], scale=1.0 / WSCALE,
                    )

                # PE: the NEXT block's tile0 + expansions lead the stream
                # (one block of lookahead gives the slow PSUM-read mu7
                # product chain a whole block of slack, and the last block
                # starts with its mu7 already in SBUF), interleaved with
                # this block's tiles so the 3 E banks recycle in pace.
                E_next = []
                d0n = None

                def lookahead(ls):
                    nonlocal d0n
                    if sg + 1 < NSG:
                        if d0n is None:
                            d0n = t0_sig(ft_bufs[sg + 1])
                        for l in ls:
                            E_next.append(expand(l, d0n))

                if sg >= 1:
                    # next block's tile0+expansions lead the PE stream
                    lookahead([0, 1, 2])
                    tile_mm(1)
                    lookahead([3, 4])
                    tile_mm(2)
                    lookahead([5, 6])
                    tile_mm(3)
                    tile_mm(4)
                else:
                    # block 0: ft1 is still held back, so the lookahead
                    # group sits mid-block to avoid HOL-blocking tiles 1-4
                    tile_mm(1)
                    tile_mm(2)
                    tile_mm(3)
                    tile_mm(4)
                    lookahead([0, 1, 2, 3, 4, 5, 6])

                # DVE: tree levels 7-8 for THIS block (mu7 ready at entry)
                mu8 = mupool.tile([128, 2 * SG], BF16, tag="mu8")
                nc.vector.tensor_mul(mu8[:, 0:SG], mu7_cur, dsg[:, 0:SG])
                nc.vector.tensor_sub(mu8[:, SG:2 * SG], mu7_cur, mu8[:, 0:SG])
                mu9 = mupool.tile([128, 4 * SG], BF16, tag="mu9")
                for j1 in range(2):
                    nc.vector.tensor_mul(
                        mu9[:, j1 * SG:(j1 + 1) * SG],
                        mu8[:, j1 * SG:(j1 + 1) * SG],
                        dsg[:, (1 + j1) * SG:(2 + j1) * SG],
                    )
                    nc.vector.tensor_sub(
                        mu9[:, (2 + j1) * SG:(3 + j1) * SG],
                        mu8[:, j1 * SG:(j1 + 1) * SG],
                        mu9[:, j1 * SG:(j1 + 1) * SG],
                    )

                # DVE: mu7 product chain for block sg+1 (for block 0 it
                # runs AFTER r9 -- its E tiles only land mid-block there)
                mu7_next = None
                if sg >= 1 and d0n is not None:
                    mu7_next = mu7_chain(E_next)

                tile_mm(5)
                tile_mm(6)
                tile_mm(7)

                # DVE: level 9 peeled into the leaf matmul --
                # y = mu9 @ P_R + (mu9 (*) d9) @ (P_L - P_R) -- so only the
                # 4 products r9 are computed; no level-9 subtractions
                r9 = mupool.tile([128, 4 * SG], BF16, tag="r9")
                for j2 in range(4):
                    nc.vector.tensor_mul(
                        r9[:, j2 * SG:(j2 + 1) * SG],
                        mu9[:, j2 * SG:(j2 + 1) * SG],
                        dsg[:, (3 + j2) * SG:(4 + j2) * SG],
                    )
                if sg < 1 and d0n is not None:
                    mu7_next = mu7_chain(E_next)

                # PE: leaf matmuls -- mu9 chunks first (ready early), then
                # the r9 chunks as their level-9 sigmoids land
                yp = py.tile([NCLS, SG], F32, tag="y")
                for i in range(8):
                    rhs = (
                        mu9[:, i * SG:(i + 1) * SG] if i < 4
                        else r9[:, (i - 4) * SG:(i - 3) * SG]
                    )
                    nc.tensor.matmul(
                        yp,
                        pp[:, i * NCLS:(i + 1) * NCLS],
                        rhs,
                        start=(i == 0), stop=(i == 7),
                    )

                # ACT copy + GpSimd SWDGE store: no queue that gates the
                # next block ever waits on leaf-matmul completion
                ysb = opool.tile([NCLS, SG], F32, tag="ysb")
                nc.scalar.copy(ysb, yp)
                nc.gpsimd.dma_start(yT[:, sg * SG:(sg + 1) * SG], ysb)
                mu7_cur = mu7_next

    nc.finalize()
    return nc


_PROGRAM = None


def _get_program():
    global _PROGRAM
    if _PROGRAM is None:
        _PROGRAM = _build_program()
    return _PROGRAM


def kernel(features, mask, W, b, pi):
    global LAST_RESULT
    features = np.asarray(features, dtype=np.float32)
    mask = np.asarray(mask)
    W = np.asarray(W, dtype=np.float32)
    b = np.asarray(b, dtype=np.float32)
    pi = np.asarray(pi, dtype=np.float32)

    # one-hot selection -> host column gather; apply slot/leaf permutations
    idx = np.argmax(mask, axis=1)
    node = _node_of_slot()
    W2p = W[:, node] * WSCALE
    W2p[:, 0] = 0.0  # slot 0 -> constant: sigmoid(0 + 30) == 1.0
    w2p_resh = np.ascontiguousarray(
        W2p.reshape(KCH, 128, NT, 128).transpose(1, 2, 0, 3).reshape(128, NT * NF)
    )
    w2p_fp8 = np.clip(w2p_resh, -240.0, 240.0).astype(ml_dtypes.float8_e4m3fn)
    b2 = b[node].astype(np.float32)
    b2[0] = 30.0
    bcols = b2.reshape(NT, 128).T                      # [128, NT]
    biases = np.ascontiguousarray(
        np.concatenate([bcols, -bcols], axis=1), dtype=np.float32
    )
    e = np.exp(pi.astype(np.float64) - pi.max(1, keepdims=True))
    probs = (e / e.sum(1, keepdims=True)).astype(np.float32)
    piP = probs[_leaf_of_row(), :].reshape(NT, 128, NCLS)
    piN = np.concatenate([piP[4:8], piP[0:4] - piP[4:8]], axis=0)
    pip_resh = np.ascontiguousarray(
        piN.transpose(1, 0, 2).reshape(128, NT * NCLS)
    ).astype(ml_dtypes.bfloat16)
    feat_fp8 = np.clip(features[:, idx], -240.0, 240.0).astype(
        ml_dtypes.float8_e4m3fn
    )
    sxp = np.ascontiguousarray(
        _expansion_mats().transpose(1, 0, 2).reshape(128, 7 * 128)
    ).astype(ml_dtypes.bfloat16)

    nc = _get_program()
    in_maps = []
    for c in range(NCORES):
        xc = feat_fp8[c * BC:(c + 1) * BC]            # [BC, NF]
        # device layout [p, sg, k, b]: feat[p, ...] = x[sg*SG+b, 128k+p]
        fdev = np.ascontiguousarray(
            xc.reshape(NSG, SG, KCH, 128).transpose(3, 0, 2, 1).reshape(128, -1)
        )
        in_maps.append(
            {"feat": fdev, "w2p": w2p_fp8, "biases": biases, "pip": pip_resh,
             "sxp": sxp}
        )
    res = run_bass_kernel_spmd(nc, in_maps, core_ids=list(range(NCORES)), **RUN_KWARGS)
    LAST_RESULT = res
    yT_full = np.concatenate([res.results[c]["yT"] for c in range(NCORES)], axis=1)
    return np.ascontiguousarray(yT_full.T)
